# revision 21
# baseline (speedup 1.0000x reference)
"""Trainium2 Bass kernel for DeformableQuantizer (vq_codebook).

Forward value of the reference:
    cb = BASE_CODEBOOK + deform_scale * delta            # [8, 3]
    per 3-elem group z: L_k = 2 z.cb_k / T               # (affine part of logits)
    E'_k = exp(L_k - C2);  B_k = exp(-|cb_k|^2/T - C3)   # C2+C3 = softmax shift
    out_c = (sum_k E'_k B_k cb_kc) / (sum_k E'_k B_k)    # softmax-weighted combine

Device pipeline, per 96-feature chunk (32 groups) and 512-token supertile,
with x host-pretransposed to feature-major fp16 (96 partitions - a multiple
of 8 so the HW DMA descriptor spread across the 16 SDMA engines engages):

    stage 1 (PE):  L[4g+k, tok] = W1^T @ xT     (fp16, 2 matmuls: code-halves)
    exp (ACT):     E = exp(L - 30)              (scalar bias; per-code bias is
                                                 folded multiplicatively into W2)
    stage 2 (PE):  [num96 | den32] = W2^T @ E   (fp32r, 2 matmuls)
    copy (DVE):    psum fp32 -> sbuf bf16
    out DMA:       [128, 4096] bf16 per supertile

Host divides num/den and re-transposes. Sharding: pure data parallel, one
batch element (4096 tokens) per NeuronCore.
"""

import itertools

import numpy as np

GROUP_DIM = 3
TEMP = 0.3
C2 = 30.0                   # scalar shift inside exp
C3 = 20.0                   # shift folded into W2 (total softmax shift 50)

N_CORES = 8
B, S, D = 8, 4096, 768
S_TILE = 512                # tokens per supertile
N_SUPER = S // S_TILE
N_CHUNKS = 8                # 96-feature chunks per 768 features
CHUNK_F = 96
CHUNK_G = 32

_BASE_CODEBOOK = np.asarray(
    list(itertools.product([-1.0, 1.0], repeat=GROUP_DIM)), dtype=np.float32
)

_CACHE: dict = {}


def _build_bass():
    import concourse.bacc as bacc
    import concourse.tile as tile
    from concourse import mybir

    f32 = mybir.dt.float32
    f32r = mybir.dt.float32r
    f16 = mybir.dt.float16
    bf16 = mybir.dt.bfloat16
    FREE = N_CHUNKS * S_TILE

    nc = bacc.Bacc()
    xin = nc.declare_dram_parameter("xin", [N_SUPER, CHUNK_F, FREE], f16, False)
    w1 = nc.declare_dram_parameter("w1", [CHUNK_F, 256], f16, False)
    w2 = nc.declare_dram_parameter("w2", [128, 256], f32r, False)
    bias = nc.declare_dram_parameter("bias", [128, 1], f32, False)
    out = nc.declare_dram_parameter("out", [N_SUPER, 128, FREE], bf16, True)

    with tile.TileContext(nc) as tc:
        with (
            tc.tile_pool(name="wpool", bufs=1) as wpool,
            tc.tile_pool(name="xpool", bufs=8) as xpool,
            tc.tile_pool(name="epool", bufs=3) as epool,
            tc.tile_pool(name="opool", bufs=3) as opool,
            tc.tile_pool(name="p1pool", bufs=1, space="PSUM") as p1pool,
            tc.tile_pool(name="p2pool", bufs=2, space="PSUM") as p2pool,
        ):
            P = 2 * S_TILE          # 1024-token pair span

            # dummy 1-element exp: forces the ~1.3us ACT_TABLE_LOAD to run
            # during the DMA warm-up instead of on the first real exp
            scratch = wpool.tile([128, 1], f32, name="scratch")
            nc.scalar.activation(
                scratch, nc.const_aps.aps[(f32, 0.0)],
                mybir.ActivationFunctionType.Exp)

            # first pair's x load goes out before the (DMA-issue-serialized)
            # weight loads so compute can start as early as possible; its two
            # matmul-halves are loaded separately so MM #1 starts sooner
            x_first = xpool.tile([CHUNK_F, P], f16, name="x_p")
            nc.sync.dma_start(
                out=x_first[:, 0:S_TILE], in_=xin[0][:, 0:S_TILE])
            w1_sb = wpool.tile([CHUNK_F, 256], f16, name="w1_sb")
            nc.sync.dma_start(out=w1_sb, in_=w1[:])
            nc.sync.dma_start(
                out=x_first[:, S_TILE:P], in_=xin[0][:, S_TILE:P])
            w2_sb = wpool.tile([128, 256], f32r, name="w2_sb")
            nc.sync.dma_start(out=w2_sb, in_=w2[:])
            bias_sb = wpool.tile([128, 1], f32, name="bias_sb")
            nc.sync.dma_start(out=bias_sb, in_=bias[:])

            def stage2(pend, split_tail=False):
                ea, eb, t, sl = pend
                ps2 = p2pool.tile([128, P], f32, name="ps2")
                o_p = opool.tile([128, P], bf16, name="o_p")
                if split_tail:
                    # final pair: token-half major, so the first half's
                    # cast+DMA overlaps the second half's matmuls
                    for h, (elo, ehi) in enumerate(
                        [(0, S_TILE), (S_TILE, P)]
                    ):
                        nc.tensor.matmul(
                            ps2[:, elo:ehi], w2_sb[:, 0:128], ea[:, elo:ehi],
                            start=True, stop=False)
                        nc.tensor.matmul(
                            ps2[:, elo:ehi], w2_sb[:, 128:256],
                            eb[:, elo:ehi], start=False, stop=True)
                        nc.vector.tensor_copy(
                            o_p[:, elo:ehi], ps2[:, elo:ehi])
                        nc.sync.dma_start(
                            out=out[t][:, sl.start + elo : sl.start + ehi],
                            in_=o_p[:, elo:ehi])
                    return
                # weight-half major so consecutive MMs share lhsT
                nc.tensor.matmul(
                    ps2[:, 0:S_TILE], w2_sb[:, 0:128], ea[:, 0:S_TILE],
                    start=True, stop=False)
                nc.tensor.matmul(
                    ps2[:, S_TILE:P], w2_sb[:, 0:128], ea[:, S_TILE:P],
                    start=True, stop=False)
                nc.tensor.matmul(
                    ps2[:, 0:S_TILE], w2_sb[:, 128:256], eb[:, 0:S_TILE],
                    start=False, stop=True)
                nc.tensor.matmul(
                    ps2[:, S_TILE:P], w2_sb[:, 128:256], eb[:, S_TILE:P],
                    start=False, stop=True)
                nc.vector.tensor_copy(o_p, ps2)
                nc.sync.dma_start(out=out[t][:, sl], in_=o_p)

            # software-pipelined: pair p's stage 1 + exp is emitted before
            # pair p-1's stage 2, so the PE never waits on the newest exp.
            # Pair-granular in/out DMAs shrink the pipeline head and tail.
            pend = None
            for t in range(N_SUPER):
                for p in range(N_CHUNKS // 2):
                    sl = slice(P * p, P * (p + 1))
                    if t == 0 and p == 0:
                        x_p = x_first
                    else:
                        x_p = xpool.tile([CHUNK_F, P], f16, name="x_p")
                        nc.sync.dma_start(out=x_p, in_=xin[t][:, sl])
                    # stage 1: per code-half, 2 N=512 matmuls sharing lhsT
                    # (PSUM bank limits a matmul to 512 fp32 output columns)
                    ps1a = p1pool.tile([128, P], f32, name="ps1a")
                    nc.tensor.matmul(
                        ps1a[:, 0:S_TILE], w1_sb[:, 0:128],
                        x_p[:, 0:S_TILE], start=True, stop=True)
                    nc.tensor.matmul(
                        ps1a[:, S_TILE:P], w1_sb[:, 0:128],
                        x_p[:, S_TILE:P], start=True, stop=True)
                    ea = epool.tile([128, P], f32r, name="ea")
                    nc.scalar.activation(
                        ea, ps1a, mybir.ActivationFunctionType.Exp,
                        bias=bias_sb)
                    ps1b = p1pool.tile([128, P], f32, name="ps1b")
                    nc.tensor.matmul(
                        ps1b[:, 0:S_TILE], w1_sb[:, 128:256],
                        x_p[:, 0:S_TILE], start=True, stop=True)
                    nc.tensor.matmul(
                        ps1b[:, S_TILE:P], w1_sb[:, 128:256],
                        x_p[:, S_TILE:P], start=True, stop=True)
                    eb = epool.tile([128, P], f32r, name="eb")
                    nc.scalar.activation(
                        eb, ps1b, mybir.ActivationFunctionType.Exp,
                        bias=bias_sb)
                    if pend is not None:
                        stage2(pend)
                    pend = (ea, eb, t, sl)
            stage2(pend, split_tail=True)
    nc.compile()
    return nc


def _weights(delta: np.ndarray, deform_scale: np.ndarray):
    cb = (_BASE_CODEBOOK + np.float32(deform_scale) * delta.astype(np.float32))
    cbn = (cb * cb).sum(1)
    bk = np.exp(-cbn / TEMP - C3).astype(np.float32)     # per-code folded bias

    w1 = np.zeros((CHUNK_F, 256), np.float32)  # [feat, (half 128) = 4g+k]
    w2 = np.zeros((128, 256), np.float32)      # [4g+k, (half 128) = 96num+32den]
    for half in range(2):
        for g in range(CHUNK_G):
            for k in range(4):
                kk = 4 * half + k
                m = 128 * half + 4 * g + k
                for c in range(GROUP_DIM):
                    w1[3 * g + c, m] = 2.0 * cb[kk, c] / TEMP
                    w2[4 * g + k, 128 * half + 3 * g + c] = cb[kk, c] * bk[kk]
                w2[4 * g + k, 128 * half + 96 + g] = bk[kk]
    return w1.astype(np.float16), w2


def _prep_core(x_core: np.ndarray) -> np.ndarray:
    # [4096, 768] -> [N_SUPER, 96, N_CHUNKS*S_TILE]; free = chunk*S_TILE + tok
    xr = x_core.reshape(N_SUPER, S_TILE, N_CHUNKS, CHUNK_F)  # t, u, c, f
    xp = np.ascontiguousarray(xr.transpose(0, 3, 2, 1)).astype(np.float16)
    return xp.reshape(N_SUPER, CHUNK_F, N_CHUNKS * S_TILE)


def _postprocess(outs: list[np.ndarray]) -> np.ndarray:
    ys = []
    for o in outs:
        o = np.asarray(o, dtype=np.float32).reshape(
            N_SUPER, 128, N_CHUNKS, S_TILE)
        num = o[:, :96].reshape(N_SUPER, CHUNK_G, 3, N_CHUNKS, S_TILE)
        den = o[:, 96:128]                       # [t, g, c, u]
        den = np.where(den == 0.0, 1.0, den)
        q = num / den[:, :, None]
        ys.append(q.transpose(0, 4, 3, 1, 2).reshape(S, D))
    return np.stack(ys).astype(np.float32)


def make_in_maps(x, delta, deform_scale):
    w1, w2 = _weights(delta, deform_scale)
    bias = np.full((128, 1), -C2, np.float32)
    return [
        {"xin": _prep_core(x[b]), "w1": w1, "w2": w2, "bias": bias}
        for b in range(N_CORES)
    ]


def kernel(x, delta, deform_scale):
    from concourse.bass_utils import run_bass_kernel_spmd

    x = np.asarray(x, dtype=np.float32)
    delta = np.asarray(delta, dtype=np.float32)
    deform_scale = np.asarray(deform_scale, dtype=np.float32)

    if "nc" not in _CACHE:
        _CACHE["nc"] = _build_bass()
    nc = _CACHE["nc"]

    in_maps = make_in_maps(x, delta, deform_scale)
    res = run_bass_kernel_spmd(nc, in_maps, core_ids=list(range(N_CORES)))
    return _postprocess([r["out"] for r in res.results])


if __name__ == "__main__":
    x = np.random.randn(B, S, D).astype(np.float32)
    delta = (np.random.randn(8, 3) * 0.1).astype(np.float32)
    ds = np.float32(0.05)
    y = kernel(x, delta, ds)
    print("out", y.shape, y.dtype)
